# revision 57
# baseline (speedup 1.0000x reference)
"""MergeAdapter (moe_routing) Trainium2 Bass kernel — transposed-output fp8 design.

Reference computation (per instance n):
    wd = sum_k prob[n,k] * w_down[k]   (D, H)     bd = sum_k prob[n,k] * b_down[k]
    wu = sum_k prob[n,k] * w_up[k]     (H, D)     bu = sum_k prob[n,k] * b_up[k]
    out[n] = x[n] + relu(x[n] @ wd.T + bd) @ wu.T + bu

Sharding: data-parallel over N=16 -> 2 instances/core on 8 cores, full expert
banks everywhere, no communication.

Design (vs the 141.5us baseline this replaces; measured ~77-88us, official
test.py run 79961ns):
  - ALL device compute happens in "transposed" space out'[h, s] instead of
    out[s, h].  Then:
      * the skip-add operand is exactly the already-loaded xT tile -> the
        8 MiB second load of x (natural layout) disappears, as do the PE
        identity-matmul skip pass and the ones-row bias pass of the old design
      * b_up becomes a per-partition scalar -> rides the epilogue bias port
      * the store is contiguous in the transposed layout; the host undoes the
        transpose (pure data movement, same as the host-side input transposes)
  - weights travel as fp8e4m3 (4 MiB instead of 8): the residual path
    tolerates fp8 easily (gate is 2e-2; the residual is ~3% of |out|;
    final rel err 3.4e-3)
  - expert-bank merging moves from 47us of DVE chains to ~7us of PE
    scaled-identity DoubleRow matmuls: merged[m,f] = sum_i p8[2kk+i] *
    bank_pair[m,i,f] accumulated over kk in PSUM, drained fp8 by ACT/DVE
  - mm2 contraction (D=256) is a single fp8e4m3 DoubleRow matmul per tile at
    0.5 cycles/row (relu1 is produced as fp8 by the ACT relu); mm1 keeps
    fp16 x (mixed fp8 lhsT x fp16 rhs runs at 1.0 cycles/row)
  - the epilogue (psum + b_up + xT skip -> fp16 out) is split A/E 50:50 so
    two engines drain PSUM concurrently:
      A: one fused scalar_tensor_tensor on DVE (psum+bias)+x
      E: x rides PE as an identity matmul into the same PSUM group, ACT
         drains with the bias on its per-partition bias port
    (Pool cannot read PSUM on TRN2, and gpsimd adds measured slow, so Pool
    only triggers the SWDGE output stores, paired 2 h-chunks per store)
  - HBM traffic: 8 (xT fp16) + 4 (w fp8) + 8 (out fp16) = 20 MiB/core vs 32;
    measured DMA-only floor ~59-68us, which is the binding resource
  - the timing loop body is UNROLLED 8x inside tc.For_i: For_i has an
    all-engine barrier per trip, so unrolling lets body k+1's loads overlap
    body k's epilogue tail (per-iteration 97 -> ~80us)
Engine budget per iteration (cost-model sim): DMA ~66us (bound), PE ~48us,
ACT ~40us, DVE ~22us, Pool ~25us.  Device-to-device measurement noise on
this shared part is +-5-10us; quiet-window best is ~74-82us.
"""
import os
import sys

for _p in ("/opt/trn_rl_repo",):
    if os.path.isdir(_p) and _p not in sys.path:
        sys.path.insert(0, _p)

import ml_dtypes
import numpy as np

import concourse.mybir as mybir
import concourse.tile as tile
from concourse import bacc
from concourse.bass_utils import run_bass_kernel_spmd

N, S, H, K, D = 16, 2048, 1024, 8, 256
NCORES = 8
NPC = N // NCORES          # instances per core
IC = H // 128              # h-chunks (contraction chunks of mm1; partition tiles of out')
OC = D // 128              # d-chunks (partition tiles of relu1; contraction of mm2)
SCW = 512                  # free-dim tile width (psum bank)
SC = S // SCW              # s-chunks
KK = K // 2                # expert pairs (DoubleRow merges 2 experts/pass)
MCW = 512                  # merge psum chunk width over the flattened bank free dim

F32 = mybir.dt.float32
F16 = mybir.dt.float16
F8 = mybir.dt.float8e4
np16 = np.float16
np8 = ml_dtypes.float8_e4m3

_CACHE: dict = {}
# epilogue routing weights per (sc,hc) tile (Pool cannot read PSUM):
#   A = fused (psum+bu)+xT on DVE            (~658ns DVE)
#   B = ACT identity+bias, then DVE 2x tt-add (~630ns ACT + ~370ns DVE)
#   Q = ACT identity+bias, then Pool tt-add   (~630ns ACT + ~1050ns Pool)
# ablate: None | "dma_only" | "compute_only"
OPTS = {"routes": {"A": 32, "E": 32}, "ablate": None,
        "banks_bufs": 1, "mdrain": "act", "ob_bufs": 8, "ilv": False,
        "fuse_sc": True, "store_pair": True, "ldq": None, "xt_full": False}


def _route_seq(n_tiles):
    w = {k: v for k, v in OPTS["routes"].items() if v > 0}
    served = {k: 0 for k in w}
    seq = []
    for _ in range(n_tiles):
        r = min(w, key=lambda k: served[k] / w[k])
        served[r] += 1
        seq.append(r)
    return seq


def _emit(nc, tc, tens, loop_t=None, unroll=1):
    (xT_d, wd8_d, wu8_d, pid_d, cst_d, eye_d, out_d) = tens
    DR = mybir.MatmulPerfMode.DoubleRow
    with (
        tc.tile_pool(name="consts", bufs=1) as consts,
        tc.tile_pool(name="banks", bufs=OPTS["banks_bufs"]) as banks,
        tc.tile_pool(name="work", bufs=1) as work,
        tc.tile_pool(name="xtp", bufs=1) as xtp,
        tc.tile_pool(name="obp", bufs=OPTS["ob_bufs"]) as obp,
        tc.tile_pool(name="tmp", bufs=5) as tmpp,
        tc.tile_pool(name="psm", bufs=2, space="PSUM") as psm,
        tc.tile_pool(name="ps1", bufs=2, space="PSUM") as ps1p,
        tc.tile_pool(name="ps2", bufs=3, space="PSUM") as ps2p,
        tc.tile_pool(name="pst", bufs=1, space="PSUM") as pstiny,
    ):
        # cst = concat([b_down (K,D) | b_up (K,H) | pkn (K,NPC)], axis=1)
        cst_t = consts.tile([K, D + H + NPC], F32, tag="cst")
        pid_t = consts.tile([128, NPC, KK, 2, 128], F8, tag="pid")
        eye_t = consts.tile([128, 128], F16, tag="eye")

        if loop_t is not None:
            assert loop_t % unroll == 0
            loop_cm = tc.For_i(0, loop_t // unroll, 1, hint_engines=tuple(
                getattr(mybir.EngineType, e)
                for e in ("PE", "DVE", "Activation", "SP", "Pool")))
        else:
            import contextlib
            loop_cm = contextlib.nullcontext()

        ABL = OPTS["ablate"]
        with loop_cm:
          for _rep in range(unroll):
            q2 = {"act": nc.scalar, "dve": nc.vector}.get(OPTS["ldq"], nc.sync)
            if ABL == "dma_only":
                # loads + equivalent-byte stores only
                wd_t = banks.tile([128, KK, 2, IC * D], F8, tag="wdall")
                wu_t = banks.tile([128, KK, 2, OC * H], F8, tag="wuall")
                nc.sync.dma_start(wd_t[:], wd8_d.ap())
                nc.sync.dma_start(wu_t[:], wu8_d.ap())
                for n in range(NPC):
                    xts = []
                    for sc in range(SC):
                        x1 = xtp.tile([128, IC, SCW], F16, tag=f"xt{n}_{sc}",
                                      name=f"xt{n}_{sc}")
                        (q2 if n == 1 else nc.sync).dma_start(
                            x1[:], xT_d.ap()[n, sc])
                        xts.append(x1)
                    for hc in range(IC):
                        for scp in range(SC // 2):
                            ob = obp.tile([128, 2 * SCW], F16, tag="ob")
                            nc.vector.tensor_copy(ob[:, 0:8],
                                                  xts[2 * scp][:, hc, 0:8])
                            nc.gpsimd.dma_start(
                                out_d.ap()[n, hc, :,
                                           scp * 2 * SCW:(scp + 1) * 2 * SCW],
                                ob[:])
                return
            SKIP_DMA = (ABL == "compute_only")

            # ---- bank loads (fp8), x chunks, tuned order ----
            wd_t = banks.tile([128, KK, 2, IC * D], F8, tag="wdall")
            wu_t = banks.tile([128, KK, 2, OC * H], F8, tag="wuall")
            xt = {}
            if OPTS["xt_full"]:
                xtf = [xtp.tile([128, IC, S], F16, tag=f"xtf{n}", name=f"xtf{n}")
                       for n in range(NPC)]

                def xsl_ap(n, sc, c):
                    return xtf[n][:, c, sc * SCW:(sc + 1) * SCW]
            else:
                for n in range(NPC):
                    for sc in range(SC):
                        xt[(n, sc)] = xtp.tile(
                            [128, IC, SCW], F16, tag=f"xt{n}_{sc}",
                            name=f"xt{n}_{sc}")

                def xsl_ap(n, sc, c):
                    return xt[(n, sc)][:, c, :]
            if not SKIP_DMA:
                nc.sync.dma_start(wd_t[:], wd8_d.ap())
            if _rep == 0:
                # consts load once per trip, after the first bank (they're
                # not needed until the bias merges / first epilogue)
                nc.sync.dma_start(cst_t[:], cst_d.ap())
                nc.sync.dma_start(pid_t[:], pid_d.ap())
                nc.sync.dma_start(eye_t[:], eye_d.ap())
            if not SKIP_DMA:
                if OPTS["xt_full"]:
                    # xT stored [n, 128, ic, S]; one plain DMA per instance
                    nc.sync.dma_start(xtf[0][:], xT_d.ap()[0])
                    nc.sync.dma_start(wu_t[:], wu8_d.ap())
                    q2.dma_start(xtf[1][:], xT_d.ap()[1])
                else:
                    nc.sync.dma_start(xt[(0, 0)][:], xT_d.ap()[0, 0])
                    q2.dma_start(xt[(1, 0)][:], xT_d.ap()[1, 0])
                    nc.sync.dma_start(wu_t[:], wu8_d.ap())
                    for sc in range(1, SC):
                        nc.sync.dma_start(xt[(0, sc)][:], xT_d.ap()[0, sc])
                        q2.dma_start(xt[(1, sc)][:], xT_d.ap()[1, sc])
            else:
                nc.gpsimd.memset(wd_t[:, 0, 0, 0:8], 0)
                nc.gpsimd.memset(wu_t[:, 0, 0, 0:8], 0)
                if OPTS["xt_full"]:
                    for n in range(NPC):
                        nc.gpsimd.memset(xtf[n][:, 0, 0:8], 0)
                else:
                    for n in range(NPC):
                        for sc in range(SC):
                            nc.gpsimd.memset(xt[(n, sc)][:, 0, 0:8], 0)

            # ---- merged biases (tiny fp32 matmuls) ----
            # mbd[:, oc*NPC+n] = merged b_down at d = oc*128+p, instance n
            # mbu[:, hc*NPC+n] = merged b_up   at h = hc*128+p, instance n
            mbd_t = work.tile([128, OC * NPC], F32, tag="mbd")
            mbu_t = work.tile([128, IC * NPC], F32, tag="mbu")
            pkn_ap = cst_t[:, D + H:D + H + NPC]
            for oc in range(OC):
                pst = pstiny.tile([128, NPC], F32, tag="pst", name="psbd")
                nc.tensor.matmul(pst[:], cst_t[:, oc * 128:(oc + 1) * 128],
                                 pkn_ap)
                nc.vector.tensor_copy(mbd_t[:, oc * NPC:(oc + 1) * NPC], pst[:])
            for hc in range(IC):
                pst = pstiny.tile([128, NPC], F32, tag="pst", name="psbu")
                nc.tensor.matmul(pst[:], cst_t[:, D + hc * 128:D + (hc + 1) * 128],
                                 pkn_ap)
                nc.vector.tensor_copy(mbu_t[:, hc * NPC:(hc + 1) * NPC], pst[:])

            # ---- expert-bank merges on PE (scaled-identity DoubleRow) ----
            # merged[m, f] = sum_kk sum_i p8[n,2kk+i] * bank[kk][m, i, f]
            wdm = [work.tile([128, IC, D], F8, tag=f"wdm{n}", name=f"wdm{n}")
                   for n in range(NPC)]
            wum = [work.tile([128, OC, H], F8, tag=f"wum{n}", name=f"wum{n}")
                   for n in range(NPC)]
            for n in range(NPC):
                for c in range(IC * D // MCW):      # wd chunks: (2 ic) x D
                    pm = psm.tile([128, 2, MCW // 2], F32, tag="psm", name="psmd")
                    for kk in range(KK):
                        nc.tensor.matmul(
                            pm[:], pid_t[:, n, kk, :, :],
                            wd_t[:, kk, :, c * MCW:(c + 1) * MCW],
                            start=(kk == 0), stop=(kk == KK - 1), perf_mode=DR)
                    if OPTS["mdrain"] == "act":
                        nc.scalar.copy(wdm[n][:, 2 * c:2 * c + 2, :], pm[:])
                    else:
                        nc.vector.tensor_copy(wdm[n][:, 2 * c:2 * c + 2, :], pm[:])
                for c in range(OC * H // MCW):      # wu chunks: (oc, h-half)
                    pm = psm.tile([128, MCW], F32, tag="psm", name="psmu")
                    for kk in range(KK):
                        nc.tensor.matmul(
                            pm[:], pid_t[:, n, kk, :, :],
                            wu_t[:, kk, :, c * MCW:(c + 1) * MCW],
                            start=(kk == 0), stop=(kk == KK - 1), perf_mode=DR)
                    if OPTS["mdrain"] == "act":
                        nc.scalar.copy(
                            wum[n][:, c // 2, (c % 2) * MCW:(c % 2) * MCW + MCW],
                            pm[:])
                    else:
                        nc.vector.tensor_copy(
                            wum[n][:, c // 2, (c % 2) * MCW:(c % 2) * MCW + MCW],
                            pm[:])

            # ---- per instance: mm1 -> relu1 (fp8), mm2 (DoubleRow) -> epilogue
            routes = _route_seq(NPC * IC * SC)
            rstate = {"t": 0}
            relu1 = [work.tile([128, OC, S], F8, tag=f"relu{n}", name=f"relu{n}")
                     for n in range(NPC)]

            def mm1_block(n, sc):
                for oc in range(OC):
                    p1 = ps1p.tile([128, SCW], F32, tag="ps1")
                    for ic in range(IC):
                        nc.tensor.matmul(
                            p1[:],
                            wdm[n][:, ic, oc * 128:(oc + 1) * 128],
                            xsl_ap(n, sc, ic),
                            start=(ic == 0), stop=(ic == IC - 1))
                    nc.scalar.activation(
                        relu1[n][:, oc, sc * SCW:(sc + 1) * SCW], p1[:],
                        mybir.ActivationFunctionType.Relu,
                        bias=mbd_t[:, oc * NPC + n:oc * NPC + n + 1], scale=1.0)

            def mm2_block(n, sc):
                ob = None
                pair = OPTS["store_pair"]
                for hc in range(IC):
                    mbu_ap = mbu_t[:, hc * NPC + n:hc * NPC + n + 1]
                    r = routes[rstate["t"]]
                    rstate["t"] += 1
                    p2 = ps2p.tile([128, SCW], F32, tag="ps2")
                    nc.tensor.matmul(
                        p2[:],
                        wum[n][:, :, hc * 128:(hc + 1) * 128],
                        relu1[n][:, :, sc * SCW:(sc + 1) * SCW],
                        start=True, stop=(r != "E"), perf_mode=DR)
                    if pair:
                        if hc % 2 == 0:
                            ob = obp.tile([128, 2, SCW], F16, tag="ob")
                        dst = ob[:, hc % 2, :]
                    else:
                        ob = obp.tile([128, SCW], F16, tag="ob")
                        dst = ob[:]
                    xsl = xsl_ap(n, sc, hc)
                    if r == "A":        # fused on DVE
                        nc.vector.scalar_tensor_tensor(
                            dst, p2[:], mbu_ap, xsl,
                            mybir.AluOpType.add, mybir.AluOpType.add)
                    elif r == "E":      # skip-add on PE, drain+bias on ACT
                        nc.tensor.matmul(
                            p2[:], eye_t[:], xsl,
                            start=False, stop=True)
                        nc.scalar.activation(
                            dst, p2[:],
                            mybir.ActivationFunctionType.Identity,
                            bias=mbu_ap, scale=1.0)
                    else:               # ACT bias-copy + {DVE,Pool} add
                        tmp = tmpp.tile([128, SCW], F16, tag="obtmp")
                        nc.scalar.activation(
                            tmp[:], p2[:],
                            mybir.ActivationFunctionType.Identity,
                            bias=mbu_ap, scale=1.0)
                        eng = nc.vector if r == "B" else nc.gpsimd
                        eng.tensor_tensor(dst, tmp[:], xsl,
                                          mybir.AluOpType.add)
                    if not SKIP_DMA:
                        if pair and hc % 2 == 1:
                            nc.gpsimd.dma_start(
                                out_d.ap()[n, hc - 1:hc + 1, :,
                                           sc * SCW:(sc + 1) * SCW]
                                .rearrange("c p s -> p c s", p=128),
                                ob[:])
                        elif not pair:
                            nc.gpsimd.dma_start(
                                out_d.ap()[n, hc, :,
                                           sc * SCW:(sc + 1) * SCW],
                                ob[:])

            if OPTS["fuse_sc"]:
                # alternate mm1/mm2 per sc: PE produces mm1(sc+1) while the
                # epilogue engines drain mm2(sc)
                for n in range(NPC):
                    for sc in range(SC):
                        mm1_block(n, sc)
                        mm2_block(n, sc)
            elif OPTS["ilv"]:
                # overlap mm2(n0)'s DVE-paced drain window with mm1(n1) on PE
                for sc in range(SC):
                    mm1_block(0, sc)
                for sc in range(SC):
                    mm2_block(0, sc)
                    mm1_block(1, sc)
                for sc in range(SC):
                    mm2_block(1, sc)
            else:
                for n in range(NPC):
                    for sc in range(SC):
                        mm1_block(n, sc)
                    for sc in range(SC):
                        mm2_block(n, sc)


def build(loop_t=None, unroll=1):
    """Build and compile the per-core NEFF. Cached per (loop_t, unroll)."""
    key = (loop_t, unroll, OPTS["ablate"], tuple(sorted(OPTS["routes"].items())),
           OPTS["banks_bufs"], OPTS["mdrain"], OPTS["ob_bufs"], OPTS["ilv"],
           OPTS["fuse_sc"], OPTS["store_pair"], OPTS["ldq"], OPTS["xt_full"])
    if key in _CACHE:
        return _CACHE[key]
    nc = bacc.Bacc("TRN2", target_bir_lowering=False, debug=False,
                   num_devices=NCORES)
    xt_shape = ([NPC, 128, IC, S] if OPTS["xt_full"]
                else [NPC, SC, 128, IC, SCW])
    tens = (
        nc.dram_tensor("xT", xt_shape, F16, kind="ExternalInput"),
        nc.dram_tensor("wd8", [128, KK, 2, IC * D], F8, kind="ExternalInput"),
        nc.dram_tensor("wu8", [128, KK, 2, OC * H], F8, kind="ExternalInput"),
        nc.dram_tensor("pid", [128, NPC, KK, 2, 128], F8, kind="ExternalInput"),
        nc.dram_tensor("cst", [K, D + H + NPC], F32, kind="ExternalInput"),
        nc.dram_tensor("eye", [128, 128], F16, kind="ExternalInput"),
        nc.dram_tensor("out", [NPC, IC, 128, S], F16, kind="ExternalOutput"),
    )
    with tile.TileContext(nc) as tc:
        _emit(nc, tc, tens, loop_t=loop_t, unroll=unroll)
    nc.compile()
    _CACHE[key] = nc
    return nc


def make_in_maps(hidden_states, prob, w_down, b_down, w_up, b_up):
    """Shard + lay out the full inputs for the 8 cores."""
    f = np.float32
    hs = np.asarray(hidden_states, dtype=f)
    prob = np.asarray(prob, dtype=f)
    # banks, interleaved expert pairs for DoubleRow:
    # wd8[p, kk, i, ic*D+d] = w_down[2kk+i, d, ic*128+p]
    wd8 = (np.asarray(w_down, f).transpose(2, 0, 1)      # (H, K, D)
           .reshape(IC, 128, KK, 2, D).transpose(1, 2, 3, 0, 4)
           .reshape(128, KK, 2, IC * D)).astype(np8)
    # wu8[p, kk, i, oc*H+h] = w_up[2kk+i, h, oc*128+p]
    wu8 = (np.asarray(w_up, f).transpose(2, 0, 1)        # (D, K, H)
           .reshape(OC, 128, KK, 2, H).transpose(1, 2, 3, 0, 4)
           .reshape(128, KK, 2, OC * H)).astype(np8)
    cst = np.concatenate([np.asarray(b_down, f), np.asarray(b_up, f)], axis=1)
    eye = np.eye(128, dtype=f)
    in_maps = []
    for c in range(NCORES):
        shard = hs[c * NPC:(c + 1) * NPC]                 # (NPC, S, H)
        p_shard = prob[c * NPC:(c + 1) * NPC]             # (NPC, K)
        if OPTS["xt_full"]:
            # xT[n, p, ic, s] = x[n, s, ic*128+p]
            xT = (shard.transpose(0, 2, 1).reshape(NPC, IC, 128, S)
                  .transpose(0, 2, 1, 3)).astype(np16)
        else:
            # xT[n, sc, p, ic, j] = x[n, sc*SCW+j, ic*128+p]
            xT = (shard.reshape(NPC, SC, SCW, IC, 128)
                  .transpose(0, 1, 4, 3, 2)).astype(np16)
        # pid[p, n, kk, i, m] = fp8(prob[n, 2kk+i]) * eye[p, m]
        p8 = p_shard.astype(np8).astype(f)                # quantized probs
        pid = (p8.reshape(1, NPC, KK, 2, 1) *
               eye.reshape(128, 1, 1, 1, 128)).astype(np8)
        in_maps.append({
            "xT": np.ascontiguousarray(xT),
            "wd8": wd8,
            "wu8": wu8,
            "pid": pid,
            "cst": np.ascontiguousarray(
                np.concatenate([cst, p_shard.T], axis=1)),
            "eye": eye.astype(np16),
        })
    return in_maps


def kernel(hidden_states, prob, w_down, b_down, w_up, b_up):
    nc = build()
    in_maps = make_in_maps(hidden_states, prob, w_down, b_down, w_up, b_up)
    res = run_bass_kernel_spmd(nc, in_maps, list(range(NCORES)))
    # out'[n, hc, p, s] -> out[n, s, hc*128+p]
    out = np.stack([res.results[c]["out"] for c in range(NCORES)], axis=0)
    out = out.reshape(N, IC * 128, S).transpose(0, 2, 1)
    return np.ascontiguousarray(out.astype(np.float32))


# revision 60
# speedup vs baseline: 1.0514x; 1.0514x over previous
"""MergeAdapter (moe_routing) Trainium2 Bass kernel — transposed-output fp8 design.

Reference computation (per instance n):
    wd = sum_k prob[n,k] * w_down[k]   (D, H)     bd = sum_k prob[n,k] * b_down[k]
    wu = sum_k prob[n,k] * w_up[k]     (H, D)     bu = sum_k prob[n,k] * b_up[k]
    out[n] = x[n] + relu(x[n] @ wd.T + bd) @ wu.T + bu

Sharding: data-parallel over N=16 -> 2 instances/core on 8 cores, full expert
banks everywhere, no communication.

Design (vs the 141.5us baseline this replaces; measured ~77-88us, official
test.py run 79961ns):
  - ALL device compute happens in "transposed" space out'[h, s] instead of
    out[s, h].  Then:
      * the skip-add operand is exactly the already-loaded xT tile -> the
        8 MiB second load of x (natural layout) disappears, as do the PE
        identity-matmul skip pass and the ones-row bias pass of the old design
      * b_up becomes a per-partition scalar -> rides the epilogue bias port
      * the store is contiguous in the transposed layout; the host undoes the
        transpose (pure data movement, same as the host-side input transposes)
  - weights travel as fp8e4m3 (4 MiB instead of 8): the residual path
    tolerates fp8 easily (gate is 2e-2; the residual is ~3% of |out|;
    final rel err 3.4e-3)
  - expert-bank merging moves from 47us of DVE chains to ~7us of PE
    scaled-identity DoubleRow matmuls: merged[m,f] = sum_i p8[2kk+i] *
    bank_pair[m,i,f] accumulated over kk in PSUM, drained fp8 by ACT/DVE
  - mm2 contraction (D=256) is a single fp8e4m3 DoubleRow matmul per tile at
    0.5 cycles/row (relu1 is produced as fp8 by the ACT relu); mm1 keeps
    fp16 x (mixed fp8 lhsT x fp16 rhs runs at 1.0 cycles/row)
  - the epilogue (psum + b_up + xT skip -> fp16 out) is split A/E 50:50 so
    two engines drain PSUM concurrently:
      A: one fused scalar_tensor_tensor on DVE (psum+bias)+x
      E: x rides PE as an identity matmul into the same PSUM group, ACT
         drains with the bias on its per-partition bias port
    (Pool cannot read PSUM on TRN2, and gpsimd adds measured slow, so Pool
    only triggers the SWDGE output stores, paired 2 h-chunks per store)
  - HBM traffic: 8 (xT fp16) + 4 (w fp8) + 8 (out fp16) = 20 MiB/core vs 32;
    measured DMA-only floor ~59-68us, which is the binding resource
  - the timing loop body is UNROLLED 8x inside tc.For_i: For_i has an
    all-engine barrier per trip, so unrolling lets body k+1's loads overlap
    body k's epilogue tail (per-iteration 97 -> ~80us)
Engine budget per iteration (cost-model sim): DMA ~66us (bound), PE ~48us,
ACT ~40us, DVE ~22us, Pool ~25us.  Device-to-device measurement noise on
this shared part is +-5-10us; quiet-window best is ~74-82us.
"""
import os
import sys

for _p in ("/opt/trn_rl_repo",):
    if os.path.isdir(_p) and _p not in sys.path:
        sys.path.insert(0, _p)

import ml_dtypes
import numpy as np

import concourse.mybir as mybir
import concourse.tile as tile
from concourse import bacc
from concourse.bass_utils import run_bass_kernel_spmd

N, S, H, K, D = 16, 2048, 1024, 8, 256
NCORES = 8
NPC = N // NCORES          # instances per core
IC = H // 128              # h-chunks (contraction chunks of mm1; partition tiles of out')
OC = D // 128              # d-chunks (partition tiles of relu1; contraction of mm2)
SCW = 512                  # free-dim tile width (psum bank)
SC = S // SCW              # s-chunks
KK = K // 2                # expert pairs (DoubleRow merges 2 experts/pass)
MCW = 512                  # merge psum chunk width over the flattened bank free dim

F32 = mybir.dt.float32
F16 = mybir.dt.float16
F8 = mybir.dt.float8e4
np16 = np.float16
np8 = ml_dtypes.float8_e4m3

_CACHE: dict = {}
# epilogue routing weights per (sc,hc) tile (Pool cannot read PSUM):
#   A = fused (psum+bu)+xT on DVE            (~658ns DVE)
#   B = ACT identity+bias, then DVE 2x tt-add (~630ns ACT + ~370ns DVE)
#   Q = ACT identity+bias, then Pool tt-add   (~630ns ACT + ~1050ns Pool)
# ablate: None | "dma_only" | "compute_only"
OPTS = {"routes": {"A": 40, "E": 24}, "ablate": None, "psm_bufs": 2, "ps2_bufs": 3,
        "banks_bufs": 1, "mdrain": "act", "ob_bufs": 8, "ilv": False,
        "fuse_sc": True, "store_pair": True, "ldq": None, "xt_full": False}


def _route_seq(n_tiles):
    w = {k: v for k, v in OPTS["routes"].items() if v > 0}
    served = {k: 0 for k in w}
    seq = []
    for _ in range(n_tiles):
        r = min(w, key=lambda k: served[k] / w[k])
        served[r] += 1
        seq.append(r)
    return seq


def _emit(nc, tc, tens, loop_t=None, unroll=1):
    (xT_d, wd8_d, wu8_d, pid_d, cst_d, eye_d, out_d) = tens
    DR = mybir.MatmulPerfMode.DoubleRow
    with (
        tc.tile_pool(name="consts", bufs=1) as consts,
        tc.tile_pool(name="banks", bufs=OPTS["banks_bufs"]) as banks,
        tc.tile_pool(name="work", bufs=1) as work,
        tc.tile_pool(name="xtp", bufs=1) as xtp,
        tc.tile_pool(name="obp", bufs=OPTS["ob_bufs"]) as obp,
        tc.tile_pool(name="tmp", bufs=5) as tmpp,
        tc.tile_pool(name="psm", bufs=OPTS["psm_bufs"], space="PSUM") as psm,
        tc.tile_pool(name="ps1", bufs=2, space="PSUM") as ps1p,
        tc.tile_pool(name="ps2", bufs=OPTS["ps2_bufs"], space="PSUM") as ps2p,
        tc.tile_pool(name="pst", bufs=1, space="PSUM") as pstiny,
    ):
        # cst = concat([b_down (K,D) | b_up (K,H) | pkn (K,NPC)], axis=1)
        cst_t = consts.tile([K, D + H + NPC], F32, tag="cst")
        pid_t = consts.tile([128, NPC, KK, 2, 128], F8, tag="pid")
        eye_t = consts.tile([128, 128], F16, tag="eye")

        if loop_t is not None:
            assert loop_t % unroll == 0
            loop_cm = tc.For_i(0, loop_t // unroll, 1, hint_engines=tuple(
                getattr(mybir.EngineType, e)
                for e in ("PE", "DVE", "Activation", "SP", "Pool")))
        else:
            import contextlib
            loop_cm = contextlib.nullcontext()

        ABL = OPTS["ablate"]
        with loop_cm:
          for _rep in range(unroll):
            q2 = {"act": nc.scalar, "dve": nc.vector}.get(OPTS["ldq"], nc.sync)
            if ABL == "dma_only":
                # loads + equivalent-byte stores only
                wd_t = banks.tile([128, KK, 2, IC * D], F8, tag="wdall")
                wu_t = banks.tile([128, KK, 2, OC * H], F8, tag="wuall")
                nc.sync.dma_start(wd_t[:], wd8_d.ap())
                nc.sync.dma_start(wu_t[:], wu8_d.ap())
                for n in range(NPC):
                    xts = []
                    for sc in range(SC):
                        x1 = xtp.tile([128, IC, SCW], F16, tag=f"xt{n}_{sc}",
                                      name=f"xt{n}_{sc}")
                        (q2 if n == 1 else nc.sync).dma_start(
                            x1[:], xT_d.ap()[n, sc])
                        xts.append(x1)
                    for hc in range(IC):
                        for scp in range(SC // 2):
                            ob = obp.tile([128, 2 * SCW], F16, tag="ob")
                            nc.vector.tensor_copy(ob[:, 0:8],
                                                  xts[2 * scp][:, hc, 0:8])
                            nc.gpsimd.dma_start(
                                out_d.ap()[n, hc, :,
                                           scp * 2 * SCW:(scp + 1) * 2 * SCW],
                                ob[:])
                return
            SKIP_DMA = (ABL == "compute_only")

            # ---- bank loads (fp8), x chunks, tuned order ----
            wd_t = banks.tile([128, KK, 2, IC * D], F8, tag="wdall")
            wu_t = banks.tile([128, KK, 2, OC * H], F8, tag="wuall")
            xt = {}
            if OPTS["xt_full"]:
                xtf = [xtp.tile([128, IC, S], F16, tag=f"xtf{n}", name=f"xtf{n}")
                       for n in range(NPC)]

                def xsl_ap(n, sc, c):
                    return xtf[n][:, c, sc * SCW:(sc + 1) * SCW]
            else:
                for n in range(NPC):
                    for sc in range(SC):
                        xt[(n, sc)] = xtp.tile(
                            [128, IC, SCW], F16, tag=f"xt{n}_{sc}",
                            name=f"xt{n}_{sc}")

                def xsl_ap(n, sc, c):
                    return xt[(n, sc)][:, c, :]
            if not SKIP_DMA:
                nc.sync.dma_start(wd_t[:], wd8_d.ap())
            if _rep == 0:
                # consts load once per trip, after the first bank (they're
                # not needed until the bias merges / first epilogue)
                nc.sync.dma_start(cst_t[:], cst_d.ap())
                nc.sync.dma_start(pid_t[:], pid_d.ap())
                nc.sync.dma_start(eye_t[:], eye_d.ap())
            if not SKIP_DMA:
                if OPTS["xt_full"]:
                    # xT stored [n, 128, ic, S]; one plain DMA per instance
                    nc.sync.dma_start(xtf[0][:], xT_d.ap()[0])
                    nc.sync.dma_start(wu_t[:], wu8_d.ap())
                    q2.dma_start(xtf[1][:], xT_d.ap()[1])
                else:
                    nc.sync.dma_start(xt[(0, 0)][:], xT_d.ap()[0, 0])
                    q2.dma_start(xt[(1, 0)][:], xT_d.ap()[1, 0])
                    nc.sync.dma_start(wu_t[:], wu8_d.ap())
                    for sc in range(1, SC):
                        nc.sync.dma_start(xt[(0, sc)][:], xT_d.ap()[0, sc])
                        q2.dma_start(xt[(1, sc)][:], xT_d.ap()[1, sc])
            else:
                nc.gpsimd.memset(wd_t[:, 0, 0, 0:8], 0)
                nc.gpsimd.memset(wu_t[:, 0, 0, 0:8], 0)
                if OPTS["xt_full"]:
                    for n in range(NPC):
                        nc.gpsimd.memset(xtf[n][:, 0, 0:8], 0)
                else:
                    for n in range(NPC):
                        for sc in range(SC):
                            nc.gpsimd.memset(xt[(n, sc)][:, 0, 0:8], 0)

            # ---- merged biases (tiny fp32 matmuls) ----
            # mbd[:, oc*NPC+n] = merged b_down at d = oc*128+p, instance n
            # mbu[:, hc*NPC+n] = merged b_up   at h = hc*128+p, instance n
            mbd_t = work.tile([128, OC * NPC], F32, tag="mbd")
            mbu_t = work.tile([128, IC * NPC], F32, tag="mbu")
            pkn_ap = cst_t[:, D + H:D + H + NPC]
            for oc in range(OC):
                pst = pstiny.tile([128, NPC], F32, tag="pst", name="psbd")
                nc.tensor.matmul(pst[:], cst_t[:, oc * 128:(oc + 1) * 128],
                                 pkn_ap)
                nc.vector.tensor_copy(mbd_t[:, oc * NPC:(oc + 1) * NPC], pst[:])
            for hc in range(IC):
                pst = pstiny.tile([128, NPC], F32, tag="pst", name="psbu")
                nc.tensor.matmul(pst[:], cst_t[:, D + hc * 128:D + (hc + 1) * 128],
                                 pkn_ap)
                nc.vector.tensor_copy(mbu_t[:, hc * NPC:(hc + 1) * NPC], pst[:])

            # ---- expert-bank merges on PE (scaled-identity DoubleRow) ----
            # merged[m, f] = sum_kk sum_i p8[n,2kk+i] * bank[kk][m, i, f]
            wdm = [work.tile([128, IC, D], F8, tag=f"wdm{n}", name=f"wdm{n}")
                   for n in range(NPC)]
            wum = [work.tile([128, OC, H], F8, tag=f"wum{n}", name=f"wum{n}")
                   for n in range(NPC)]
            for n in range(NPC):
                for c in range(IC * D // MCW):      # wd chunks: (2 ic) x D
                    pm = psm.tile([128, 2, MCW // 2], F32, tag="psm", name="psmd")
                    for kk in range(KK):
                        nc.tensor.matmul(
                            pm[:], pid_t[:, n, kk, :, :],
                            wd_t[:, kk, :, c * MCW:(c + 1) * MCW],
                            start=(kk == 0), stop=(kk == KK - 1), perf_mode=DR)
                    if OPTS["mdrain"] == "act":
                        nc.scalar.copy(wdm[n][:, 2 * c:2 * c + 2, :], pm[:])
                    else:
                        nc.vector.tensor_copy(wdm[n][:, 2 * c:2 * c + 2, :], pm[:])
                for c in range(OC * H // MCW):      # wu chunks: (oc, h-half)
                    pm = psm.tile([128, MCW], F32, tag="psm", name="psmu")
                    for kk in range(KK):
                        nc.tensor.matmul(
                            pm[:], pid_t[:, n, kk, :, :],
                            wu_t[:, kk, :, c * MCW:(c + 1) * MCW],
                            start=(kk == 0), stop=(kk == KK - 1), perf_mode=DR)
                    if OPTS["mdrain"] == "act":
                        nc.scalar.copy(
                            wum[n][:, c // 2, (c % 2) * MCW:(c % 2) * MCW + MCW],
                            pm[:])
                    else:
                        nc.vector.tensor_copy(
                            wum[n][:, c // 2, (c % 2) * MCW:(c % 2) * MCW + MCW],
                            pm[:])

            # ---- per instance: mm1 -> relu1 (fp8), mm2 (DoubleRow) -> epilogue
            routes = _route_seq(NPC * IC * SC)
            rstate = {"t": 0}
            relu1 = [work.tile([128, OC, S], F8, tag=f"relu{n}", name=f"relu{n}")
                     for n in range(NPC)]

            def mm1_block(n, sc):
                for oc in range(OC):
                    p1 = ps1p.tile([128, SCW], F32, tag="ps1")
                    for ic in range(IC):
                        nc.tensor.matmul(
                            p1[:],
                            wdm[n][:, ic, oc * 128:(oc + 1) * 128],
                            xsl_ap(n, sc, ic),
                            start=(ic == 0), stop=(ic == IC - 1))
                    nc.scalar.activation(
                        relu1[n][:, oc, sc * SCW:(sc + 1) * SCW], p1[:],
                        mybir.ActivationFunctionType.Relu,
                        bias=mbd_t[:, oc * NPC + n:oc * NPC + n + 1], scale=1.0)

            def mm2_block(n, sc):
                ob = None
                pair = OPTS["store_pair"]
                for hc in range(IC):
                    mbu_ap = mbu_t[:, hc * NPC + n:hc * NPC + n + 1]
                    r = routes[rstate["t"]]
                    rstate["t"] += 1
                    p2 = ps2p.tile([128, SCW], F32, tag="ps2")
                    nc.tensor.matmul(
                        p2[:],
                        wum[n][:, :, hc * 128:(hc + 1) * 128],
                        relu1[n][:, :, sc * SCW:(sc + 1) * SCW],
                        start=True, stop=(r != "E"), perf_mode=DR)
                    if pair:
                        if hc % 2 == 0:
                            ob = obp.tile([128, 2, SCW], F16, tag="ob")
                        dst = ob[:, hc % 2, :]
                    else:
                        ob = obp.tile([128, SCW], F16, tag="ob")
                        dst = ob[:]
                    xsl = xsl_ap(n, sc, hc)
                    if r == "A":        # fused on DVE
                        nc.vector.scalar_tensor_tensor(
                            dst, p2[:], mbu_ap, xsl,
                            mybir.AluOpType.add, mybir.AluOpType.add)
                    elif r == "E":      # skip-add on PE, drain+bias on ACT
                        nc.tensor.matmul(
                            p2[:], eye_t[:], xsl,
                            start=False, stop=True)
                        nc.scalar.activation(
                            dst, p2[:],
                            mybir.ActivationFunctionType.Identity,
                            bias=mbu_ap, scale=1.0)
                    else:               # ACT bias-copy + {DVE,Pool} add
                        tmp = tmpp.tile([128, SCW], F16, tag="obtmp")
                        nc.scalar.activation(
                            tmp[:], p2[:],
                            mybir.ActivationFunctionType.Identity,
                            bias=mbu_ap, scale=1.0)
                        eng = nc.vector if r == "B" else nc.gpsimd
                        eng.tensor_tensor(dst, tmp[:], xsl,
                                          mybir.AluOpType.add)
                    if not SKIP_DMA:
                        if pair and hc % 2 == 1:
                            nc.gpsimd.dma_start(
                                out_d.ap()[n, hc - 1:hc + 1, :,
                                           sc * SCW:(sc + 1) * SCW]
                                .rearrange("c p s -> p c s", p=128),
                                ob[:])
                        elif not pair:
                            nc.gpsimd.dma_start(
                                out_d.ap()[n, hc, :,
                                           sc * SCW:(sc + 1) * SCW],
                                ob[:])

            if OPTS["fuse_sc"]:
                # alternate mm1/mm2 per sc: PE produces mm1(sc+1) while the
                # epilogue engines drain mm2(sc)
                for n in range(NPC):
                    for sc in range(SC):
                        mm1_block(n, sc)
                        mm2_block(n, sc)
            elif OPTS["ilv"]:
                # overlap mm2(n0)'s DVE-paced drain window with mm1(n1) on PE
                for sc in range(SC):
                    mm1_block(0, sc)
                for sc in range(SC):
                    mm2_block(0, sc)
                    mm1_block(1, sc)
                for sc in range(SC):
                    mm2_block(1, sc)
            else:
                for n in range(NPC):
                    for sc in range(SC):
                        mm1_block(n, sc)
                    for sc in range(SC):
                        mm2_block(n, sc)


def build(loop_t=None, unroll=1):
    """Build and compile the per-core NEFF. Cached per (loop_t, unroll)."""
    key = (loop_t, unroll, OPTS["ablate"], tuple(sorted(OPTS["routes"].items())),
           OPTS["banks_bufs"], OPTS["mdrain"], OPTS["ob_bufs"], OPTS["ilv"],
           OPTS["fuse_sc"], OPTS["store_pair"], OPTS["ldq"], OPTS["xt_full"],
           OPTS["psm_bufs"], OPTS["ps2_bufs"])
    if key in _CACHE:
        return _CACHE[key]
    nc = bacc.Bacc("TRN2", target_bir_lowering=False, debug=False,
                   num_devices=NCORES)
    xt_shape = ([NPC, 128, IC, S] if OPTS["xt_full"]
                else [NPC, SC, 128, IC, SCW])
    tens = (
        nc.dram_tensor("xT", xt_shape, F16, kind="ExternalInput"),
        nc.dram_tensor("wd8", [128, KK, 2, IC * D], F8, kind="ExternalInput"),
        nc.dram_tensor("wu8", [128, KK, 2, OC * H], F8, kind="ExternalInput"),
        nc.dram_tensor("pid", [128, NPC, KK, 2, 128], F8, kind="ExternalInput"),
        nc.dram_tensor("cst", [K, D + H + NPC], F32, kind="ExternalInput"),
        nc.dram_tensor("eye", [128, 128], F16, kind="ExternalInput"),
        nc.dram_tensor("out", [NPC, IC, 128, S], F16, kind="ExternalOutput"),
    )
    with tile.TileContext(nc) as tc:
        _emit(nc, tc, tens, loop_t=loop_t, unroll=unroll)
    nc.compile()
    _CACHE[key] = nc
    return nc


def make_in_maps(hidden_states, prob, w_down, b_down, w_up, b_up):
    """Shard + lay out the full inputs for the 8 cores."""
    f = np.float32
    hs = np.asarray(hidden_states, dtype=f)
    prob = np.asarray(prob, dtype=f)
    # banks, interleaved expert pairs for DoubleRow:
    # wd8[p, kk, i, ic*D+d] = w_down[2kk+i, d, ic*128+p]
    wd8 = (np.asarray(w_down, f).transpose(2, 0, 1)      # (H, K, D)
           .reshape(IC, 128, KK, 2, D).transpose(1, 2, 3, 0, 4)
           .reshape(128, KK, 2, IC * D)).astype(np8)
    # wu8[p, kk, i, oc*H+h] = w_up[2kk+i, h, oc*128+p]
    wu8 = (np.asarray(w_up, f).transpose(2, 0, 1)        # (D, K, H)
           .reshape(OC, 128, KK, 2, H).transpose(1, 2, 3, 0, 4)
           .reshape(128, KK, 2, OC * H)).astype(np8)
    cst = np.concatenate([np.asarray(b_down, f), np.asarray(b_up, f)], axis=1)
    eye = np.eye(128, dtype=f)
    in_maps = []
    for c in range(NCORES):
        shard = hs[c * NPC:(c + 1) * NPC]                 # (NPC, S, H)
        p_shard = prob[c * NPC:(c + 1) * NPC]             # (NPC, K)
        if OPTS["xt_full"]:
            # xT[n, p, ic, s] = x[n, s, ic*128+p]
            xT = (shard.transpose(0, 2, 1).reshape(NPC, IC, 128, S)
                  .transpose(0, 2, 1, 3)).astype(np16)
        else:
            # xT[n, sc, p, ic, j] = x[n, sc*SCW+j, ic*128+p]
            xT = (shard.reshape(NPC, SC, SCW, IC, 128)
                  .transpose(0, 1, 4, 3, 2)).astype(np16)
        # pid[p, n, kk, i, m] = fp8(prob[n, 2kk+i]) * eye[p, m]
        p8 = p_shard.astype(np8).astype(f)                # quantized probs
        pid = (p8.reshape(1, NPC, KK, 2, 1) *
               eye.reshape(128, 1, 1, 1, 128)).astype(np8)
        in_maps.append({
            "xT": np.ascontiguousarray(xT),
            "wd8": wd8,
            "wu8": wu8,
            "pid": pid,
            "cst": np.ascontiguousarray(
                np.concatenate([cst, p_shard.T], axis=1)),
            "eye": eye.astype(np16),
        })
    return in_maps


def kernel(hidden_states, prob, w_down, b_down, w_up, b_up):
    nc = build()
    in_maps = make_in_maps(hidden_states, prob, w_down, b_down, w_up, b_up)
    res = run_bass_kernel_spmd(nc, in_maps, list(range(NCORES)))
    # out'[n, hc, p, s] -> out[n, s, hc*128+p]
    out = np.stack([res.results[c]["out"] for c in range(NCORES)], axis=0)
    out = out.reshape(N, IC * 128, S).transpose(0, 2, 1)
    return np.ascontiguousarray(out.astype(np.float32))
